# revision 28
# baseline (speedup 1.0000x reference)
"""Dynamic per-pixel depthwise 3x3 conv (DYDConv2d) on 8 Trainium2 cores.

Full-tensor contract:
    input : (8, 64, 128, 128) f32
    weight: (8, 64, 3, 3, 128, 128) f32   -- one 3x3 filter per (b, c, oh, ow)
    out   : (8, 64, 128, 128) f32
    out[b,c,oh,ow] = sum_{i,j} xpad[b,c,oh+i,ow+j] * weight[b,c,i,j,oh,ow]
    (stride 1, pad 1)

Sharding: data-parallel over batch B=8 -> one sample per NeuronCore.

fp16 throughout on device (harness gate is rel_err < 2e-2; measured fp16
error ~9.1e-4): halves the ~38 MB/core weight stream that is the DMA
roofline term AND unlocks the DVE 2x_1P packed perf mode, halving the
vector-engine time of the 17 tensor_tensor ops per output element. Both
roofline terms drop 2x vs the fp32 kernel (measured floors per pass:
DVE ~70 us, HBM DMA ~63-69 us; full pass ~70-74 us vs ~147 us true fp32).

Per-core layout: 128 SBUF partitions = (channel c) x (H-half hf), p =
c*2 + hf. Each partition holds TWO 66x130 fp16 zero-padded slabs of its
half-image: slab A (natural) serving column taps j=0 (byte offset 0) and
j=2 (offset 4), and slab B (= A shifted left one column) serving j=1 at
offset 0. Without B, the j=1 taps start 2 bytes into a 4-byte word and the
DVE silently drops those ops from 2x_1P to 1x packing. Both slabs ship from
the host (xmode="dual"; on-device B builds were tried: GPSIMD copy loses to
DVE port contention, DVE copy adds to the critical DVE chain, SBUF->SBUF
DMA timed ~equal but showed one silent-corruption flake). The full
half-image is one rt=64-row chunk: the 9 weight tiles (2 MB DMAs) stream in
groups of 2, double-buffered, while the DVE runs the 17-op mult/add chain
(FD = 8192). Engine split: weights + out-store on the sync HWDGE queue,
x load on the scalar queue. kernel() spot-checks 512 random outputs on the
host and reruns the device pass on mismatch (cold-NEFF flakes observed).
"""

import numpy as np

import concourse.bacc as bacc
import concourse.mybir as mybir
from concourse.bass_utils import run_bass_kernel_spmd
from concourse.tile import TileContext

B, C, H, W = 8, 64, 128, 128
KH, KW = 3, 3
HALF = H // 2  # rows per half-image (one partition group)
SLAB_R, SLAB_C = HALF + 2, W + 2  # 66 x 130 padded slab per partition

RT = 64   # output rows per chunk (per half); 64 = one chunk per pass
GRP = 2   # weight tiles streamed per group
XMODE = "dual"  # "dual": host ships A+B slabs | "sbufb"/"gpsb"/"dveb":
                # host ships A only, B built on-device (SBUF-SBUF DMA /
                # GPSIMD copy / DVE copy). sbufb saves 2.2 MB/pass of HBM
                # reads and timed ~72 us with a tighter spread, but showed
                # one silent-corruption flake; dual has a perfect
                # correctness record, so dual ships.

_F16 = mybir.dt.float16


def _emit(nc, tc, xs, w, o, rep=1, rt=None, grp=None, mode="full",
          xsplit=False, osync=False, tmpb=1, xmode=None, bq="sync", xq="scalar"):
    """Per-core program. xs:[128, 2*66*130] f16, w:[64,3,3,128,128] f16,
    o:[64,128,128] f16.

    rep > 1 repeats the complete pass (x load included) back-to-back in one
    program -- used only for steady-state timing via differencing.
    mode: "full" | "dma" (no DVE ops) | "compute" (no DMAs) -- for
    roofline decomposition.
    """
    rt = RT if rt is None else rt
    grp = GRP if grp is None else grp
    wv = w.rearrange("c kh kw (hf r) ww -> c hf (kh kw) r ww", hf=2)
    ov = o.rearrange("c (hf r) ww -> (c hf) r ww", hf=2)

    with tc.tile_pool(name="work", bufs=2) as pool:
        if mode == "compute":
            # DMAs only in a setup prologue; reps are pure DVE work.
            # Rep-differencing subtracts the prologue.
            xbuf = pool.tile([128, 2, SLAB_R, SLAB_C], _F16, name="xbuf", bufs=1)
            nc.scalar.dma_start(
                out=xbuf[:].rearrange("p a r cc -> p (a r cc)"), in_=xs[:]
            )
            wts = [
                pool.tile([128, rt, W], _F16, name=f"wc{t}", bufs=1)
                for t in range(3)
            ]
            for t in range(3):
                nc.sync.dma_start(out=wts[t][:], in_=wv[:, :, t, 0:rt, :])
            for _r in range(rep):
                for k in range(HALF // rt):
                    r0 = k * rt
                    acc = pool.tile([128, rt, W], _F16, name="acc")
                    tmp = pool.tile([128, rt, W], _F16, name="tmp", bufs=1)
                    first = True
                    for t in range(KH * KW):
                        wt = wts[t % 3]
                        if first:
                            nc.vector.tensor_tensor(
                                acc[:], _xtap(xbuf, r0, rt, t), wt[:],
                                mybir.AluOpType.mult,
                            )
                            first = False
                        else:
                            nc.vector.tensor_tensor(
                                tmp[:], _xtap(xbuf, r0, rt, t), wt[:],
                                mybir.AluOpType.mult,
                            )
                            nc.vector.tensor_tensor(
                                acc[:], acc[:], tmp[:], mybir.AluOpType.add
                            )
                    nc.scalar.dma_start(
                        out=ov[:, r0 : r0 + rt, :], in_=acc[:]
                    )
            return
        xmode = XMODE if xmode is None else xmode
        for _r in range(rep):
            # double-buffered slab: next rep's x load overlaps the previous
            # rep's tail instead of serializing behind it
            xbuf = pool.tile([128, 2, SLAB_R, SLAB_C], _F16, name="xbuf")
            xeng = nc.sync if xq == "sync" else nc.scalar
            if xmode == "dual":
                xv = xs.rearrange("p (a r cc) -> p a r cc", a=2, r=SLAB_R)
                if xsplit:
                    # split by half-slab rows so chunk 0 can start once its
                    # region lands (subtile deps), not after the full 4.4MB
                    hr = SLAB_R // 2 + 2
                    for ab in range(2):
                        xeng.dma_start(
                            out=xbuf[:, ab, 0:hr, :], in_=xv[:, ab, 0:hr, :]
                        )
                        xeng.dma_start(
                            out=xbuf[:, ab, hr:SLAB_R, :],
                            in_=xv[:, ab, hr:SLAB_R, :],
                        )
                else:
                    xeng.dma_start(
                        out=xbuf[:].rearrange("p a r cc -> p (a r cc)"),
                        in_=xs[:],
                    )
            else:
                # host ships slab A only; build B = A shifted one column
                nc.scalar.dma_start(
                    out=xbuf[:, 0, :, :].rearrange("p r cc -> p (r cc)"),
                    in_=xs[:],
                )
                bdst = xbuf[:, 1, :, 0 : SLAB_C - 1]
                bsrc = xbuf[:, 0, :, 1:SLAB_C]
                if xmode == "sbufb":
                    beng = nc.scalar if bq == "scalar" else nc.sync
                    beng.dma_start(out=bdst, in_=bsrc)
                elif xmode == "gpsb":
                    nc.gpsimd.tensor_copy(bdst, bsrc)
                elif xmode == "dveb":
                    nc.vector.tensor_copy(bdst, bsrc)
                else:
                    raise ValueError(xmode)
            _emit_pass(nc, pool, xbuf, wv, ov, rt=rt, grp=grp, mode=mode,
                       osync=osync, tmpb=tmpb)


def _xtap(xbuf, r0, rt, t):
    i, j = divmod(t, KW)
    ab, col = (1, 0) if j == 1 else (0, j)
    return xbuf[:, ab, r0 + i : r0 + i + rt, col : col + W]


def _emit_pass(nc, pool, xbuf, wv, ov, rt, grp, mode="full", osync=False,
               tmpb=1):
    for k in range(HALF // rt):
        r0 = k * rt
        if mode == "dma":
            for t in range(KH * KW):
                wt = pool.tile([128, rt, W], _F16, name=f"wg{t % grp}")
                nc.sync.dma_start(out=wt[:], in_=wv[:, :, t, r0 : r0 + rt, :])
            nc.scalar.dma_start(
                out=ov[:, r0 : r0 + rt, :], in_=xbuf[:, 0, r0 : r0 + rt, 0:W]
            )
            continue
        acc = pool.tile([128, rt, W], _F16, name="acc")
        tmp = pool.tile([128, rt, W], _F16, name="tmp", bufs=tmpb)
        first = True
        for g0 in range(0, KH * KW, grp):
            wts = []
            for t in range(g0, min(g0 + grp, KH * KW)):
                wt = pool.tile([128, rt, W], _F16, name=f"wg{t - g0}")
                if mode != "compute":
                    nc.sync.dma_start(
                        out=wt[:], in_=wv[:, :, t, r0 : r0 + rt, :]
                    )
                wts.append((t, wt))
            for t, wt in wts:
                if first:
                    nc.vector.tensor_tensor(
                        acc[:], _xtap(xbuf, r0, rt, t), wt[:],
                        mybir.AluOpType.mult,
                    )
                    first = False
                else:
                    nc.vector.tensor_tensor(
                        tmp[:], _xtap(xbuf, r0, rt, t), wt[:],
                        mybir.AluOpType.mult,
                    )
                    nc.vector.tensor_tensor(
                        acc[:], acc[:], tmp[:], mybir.AluOpType.add
                    )
        if mode != "compute":
            eng = nc.sync if osync else nc.scalar
            eng.dma_start(out=ov[:, r0 : r0 + rt, :], in_=acc[:])


def build_program(rep=1, rt=None, grp=None, mode="full", xsplit=False,
                  osync=True, tmpb=1, xmode=None, bq="sync", xq="scalar"):
    xmode = XMODE if xmode is None else xmode
    nc = bacc.Bacc(
        "TRN2",
        target_bir_lowering=False,
        debug=False,
        enable_asserts=False,
        num_devices=8,
    )
    nslab = 2 if xmode == "dual" else 1
    xs = nc.dram_tensor(
        "xs", [128, nslab * SLAB_R * SLAB_C], _F16, kind="ExternalInput"
    ).ap()
    w = nc.dram_tensor("w", [C, KH, KW, H, W], _F16, kind="ExternalInput").ap()
    o = nc.dram_tensor("o", [C, H, W], _F16, kind="ExternalOutput").ap()
    with TileContext(nc) as tc:
        _emit(nc, tc, xs, w, o, rep=rep, rt=rt, grp=grp, mode=mode,
              xsplit=xsplit, osync=osync, tmpb=tmpb, xmode=xmode, bq=bq,
              xq=xq)
    nc.compile()
    return nc


def make_slab(x_one, xmode=None):
    """Host-side fp16 slab(s) for one sample: [64,128,128] ->
    [128, {1,2}*66*130].

    Partition p = c*2 + hf holds rows hf*64-1 .. hf*64+64 of channel c
    (zero-padded at the image border) in a 66x130 col-padded layout: slab A
    natural and (xmode=="dual" only) slab B shifted left one column (so the
    j=1 column tap starts 4-byte-aligned and keeps the DVE in 2x_1P packed
    mode; the other xmodes build B on-device).
    """
    xmode = XMODE if xmode is None else xmode
    dual = xmode == "dual"
    nslab = 2 if dual else 1
    xh = np.ascontiguousarray(x_one).astype(np.float16)
    slab = np.zeros((C, 2, nslab, SLAB_R, SLAB_C), dtype=np.float16)
    # half 0: slab rows 1..65 <- x rows 0..64 (row 0 stays zero: top pad)
    slab[:, 0, 0, 1 : HALF + 2, 1 : W + 1] = xh[:, 0 : HALF + 1, :]
    # half 1: slab rows 0..64 <- x rows 63..127 (row 65 stays zero: bottom pad)
    slab[:, 1, 0, 0 : HALF + 1, 1 : W + 1] = xh[:, HALF - 1 : H, :]
    if dual:
        # slab B = slab A shifted left one column
        slab[:, :, 1, :, 0 : SLAB_C - 1] = slab[:, :, 0, :, 1:SLAB_C]
    return slab.reshape(128, nslab * SLAB_R * SLAB_C)


def host_in_full(input, weight, xmode=None):
    """Full-batch host-side input dict keyed by DRAM tensor names, with each
    array laid out as the concat over cores along axis 0 (for shard_map)."""
    return {
        "xs": np.concatenate(
            [make_slab(input[b], xmode) for b in range(B)], axis=0
        ),
        "w": np.ascontiguousarray(weight).astype(np.float16).reshape(
            B * C, KH, KW, H, W
        ),
    }


_CACHE = {}


def _spot_check(out, input, weight, n=512, seed=0):
    """Host-check a random subset of outputs against the fp16-rounded math.

    Catches silent device corruption (cold-NEFF flakes produce garbage far
    outside the ~1e-3 fp16 rounding envelope).
    """
    rng = np.random.default_rng(seed)
    bb = rng.integers(0, B, n)
    cc = rng.integers(0, C, n)
    hh = rng.integers(0, H, n)
    ww = rng.integers(0, W, n)
    xh = input.astype(np.float16)
    whh = weight.astype(np.float16)
    ref = np.zeros(n, np.float64)
    for i in range(KH):
        for j in range(KW):
            h_in = hh + i - 1
            w_in = ww + j - 1
            valid = (h_in >= 0) & (h_in < H) & (w_in >= 0) & (w_in < W)
            xv = np.where(
                valid,
                xh[bb, cc, np.clip(h_in, 0, H - 1), np.clip(w_in, 0, W - 1)],
                np.float16(0),
            ).astype(np.float64)
            ref += xv * whh[bb, cc, i, j, hh, ww].astype(np.float64)
    got = out[bb, cc, hh, ww].astype(np.float64)
    scale = max(np.abs(ref).max(), 1.0)
    return np.abs(got - ref).max() / scale < 5e-3


def kernel(input, weight, _trace=False):
    input = np.asarray(input, dtype=np.float32)
    weight = np.asarray(weight, dtype=np.float32)
    assert input.shape == (B, C, H, W), input.shape
    assert weight.shape == (B, C, KH, KW, H, W), weight.shape

    if "nc" not in _CACHE:
        _CACHE["nc"] = build_program()
    nc = _CACHE["nc"]

    wh = weight.astype(np.float16)
    in_maps = [
        {"xs": make_slab(input[b]), "w": np.ascontiguousarray(wh[b])}
        for b in range(B)
    ]

    def run_once():
        res = run_bass_kernel_spmd(
            nc, in_maps, core_ids=list(range(B)), trace=_trace
        )
        _CACHE["last_result"] = res
        return np.stack([res.results[b]["o"] for b in range(B)], axis=0)

    # Retry both on exceptions (transient NRT/axon hiccups, e.g.
    # NRT_EXEC_UNIT_UNRECOVERABLE on a cold device) and on silent
    # corruption (spot-check failure) -- both observed, both clear on
    # rerun.
    import time as _time

    out = None
    for attempt in range(3):
        try:
            out = run_once()
        except Exception:
            if attempt == 2:
                raise
            _time.sleep(3)
            continue
        if _spot_check(out, input, weight):
            break
        _time.sleep(1)
    return out.astype(np.float32)


# revision 29
# speedup vs baseline: 1.1358x; 1.1358x over previous
"""Dynamic per-pixel depthwise 3x3 conv (DYDConv2d) on 8 Trainium2 cores.

Full-tensor contract:
    input : (8, 64, 128, 128) f32
    weight: (8, 64, 3, 3, 128, 128) f32   -- one 3x3 filter per (b, c, oh, ow)
    out   : (8, 64, 128, 128) f32
    out[b,c,oh,ow] = sum_{i,j} xpad[b,c,oh+i,ow+j] * weight[b,c,i,j,oh,ow]
    (stride 1, pad 1)

Sharding: data-parallel over batch B=8 -> one sample per NeuronCore.

fp16 throughout on device (harness gate is rel_err < 2e-2; measured fp16
error ~9.1e-4): halves the ~38 MB/core weight stream that is the DMA
roofline term AND unlocks the DVE 2x_1P packed perf mode, halving the
vector-engine time of the 17 tensor_tensor ops per output element. Both
roofline terms drop 2x vs the fp32 kernel (measured floors per pass:
DVE ~70 us, HBM DMA ~63-69 us; full pass ~70-74 us vs ~147 us true fp32).

Per-core layout: 128 SBUF partitions = (channel c) x (H-half hf), p =
c*2 + hf. Each partition holds TWO 66x130 fp16 zero-padded slabs of its
half-image: slab A (natural) serving column taps j=0 (byte offset 0) and
j=2 (offset 4), and slab B (= A shifted left one column) serving j=1 at
offset 0. Without B, the j=1 taps start 2 bytes into a 4-byte word and the
DVE silently drops those ops from 2x_1P to 1x packing. Both slabs ship from
the host (xmode="dual"; on-device B builds were tried: GPSIMD copy loses to
DVE port contention, DVE copy adds to the critical DVE chain, SBUF->SBUF
DMA timed ~equal but showed one silent-corruption flake). The full
half-image is one rt=64-row chunk: the 9 weight tiles (2 MB DMAs) stream in
groups of 2, double-buffered, while the DVE runs the 17-op mult/add chain
(FD = 8192). Engine split: weights + out-store on the sync HWDGE queue,
x load on the scalar queue. kernel() spot-checks 512 random outputs on the
host and reruns the device pass on mismatch (cold-NEFF flakes observed).
"""

import numpy as np

import concourse.bacc as bacc
import concourse.mybir as mybir
from concourse.bass_utils import run_bass_kernel_spmd
from concourse.tile import TileContext

B, C, H, W = 8, 64, 128, 128
KH, KW = 3, 3
HALF = H // 2  # rows per half-image (one partition group)
SLAB_R, SLAB_C = HALF + 2, W + 2  # 66 x 130 padded slab per partition

RT = 64   # output rows per chunk (per half); 64 = one chunk per pass
GRP = 2   # weight tiles streamed per group
XMODE = "dual"  # "dual": host ships A+B slabs | "sbufb"/"gpsb"/"dveb":
                # host ships A only, B built on-device (SBUF-SBUF DMA /
                # GPSIMD copy / DVE copy). sbufb saves 2.2 MB/pass of HBM
                # reads and timed ~72 us with a tighter spread, but showed
                # one silent-corruption flake; dual has a perfect
                # correctness record, so dual ships.

_F16 = mybir.dt.float16


def _emit(nc, tc, xs, w, o, rep=1, rt=None, grp=None, mode="full",
          xsplit=False, osync=False, tmpb=1, xmode=None, bq="sync",
          xq="scalar", oq=None, wsplit=False):
    """Per-core program. xs:[128, 2*66*130] f16, w:[64,3,3,128,128] f16,
    o:[64,128,128] f16.

    rep > 1 repeats the complete pass (x load included) back-to-back in one
    program -- used only for steady-state timing via differencing.
    mode: "full" | "dma" (no DVE ops) | "compute" (no DMAs) -- for
    roofline decomposition.
    """
    rt = RT if rt is None else rt
    grp = GRP if grp is None else grp
    wv = w.rearrange("c kh kw (hf r) ww -> c hf (kh kw) r ww", hf=2)
    ov = o.rearrange("c (hf r) ww -> (c hf) r ww", hf=2)

    with tc.tile_pool(name="work", bufs=2) as pool:
        if mode == "compute":
            # DMAs only in a setup prologue; reps are pure DVE work.
            # Rep-differencing subtracts the prologue.
            xbuf = pool.tile([128, 2, SLAB_R, SLAB_C], _F16, name="xbuf", bufs=1)
            nc.scalar.dma_start(
                out=xbuf[:].rearrange("p a r cc -> p (a r cc)"), in_=xs[:]
            )
            wts = [
                pool.tile([128, rt, W], _F16, name=f"wc{t}", bufs=1)
                for t in range(3)
            ]
            for t in range(3):
                nc.sync.dma_start(out=wts[t][:], in_=wv[:, :, t, 0:rt, :])
            for _r in range(rep):
                for k in range(HALF // rt):
                    r0 = k * rt
                    acc = pool.tile([128, rt, W], _F16, name="acc")
                    tmp = pool.tile([128, rt, W], _F16, name="tmp", bufs=1)
                    first = True
                    for t in range(KH * KW):
                        wt = wts[t % 3]
                        if first:
                            nc.vector.tensor_tensor(
                                acc[:], _xtap(xbuf, r0, rt, t), wt[:],
                                mybir.AluOpType.mult,
                            )
                            first = False
                        else:
                            nc.vector.tensor_tensor(
                                tmp[:], _xtap(xbuf, r0, rt, t), wt[:],
                                mybir.AluOpType.mult,
                            )
                            nc.vector.tensor_tensor(
                                acc[:], acc[:], tmp[:], mybir.AluOpType.add
                            )
                    nc.scalar.dma_start(
                        out=ov[:, r0 : r0 + rt, :], in_=acc[:]
                    )
            return
        xmode = XMODE if xmode is None else xmode
        for _r in range(rep):
            # double-buffered slab: next rep's x load overlaps the previous
            # rep's tail instead of serializing behind it
            xbuf = pool.tile([128, 2, SLAB_R, SLAB_C], _F16, name="xbuf")
            xeng = {"sync": nc.sync, "scalar": nc.scalar,
                    "gpsimd": nc.gpsimd}[xq]
            if xmode == "dual":
                xv = xs.rearrange("p (a r cc) -> p a r cc", a=2, r=SLAB_R)
                if xsplit:
                    # split by half-slab rows so chunk 0 can start once its
                    # region lands (subtile deps), not after the full 4.4MB
                    hr = SLAB_R // 2 + 2
                    for ab in range(2):
                        xeng.dma_start(
                            out=xbuf[:, ab, 0:hr, :], in_=xv[:, ab, 0:hr, :]
                        )
                        xeng.dma_start(
                            out=xbuf[:, ab, hr:SLAB_R, :],
                            in_=xv[:, ab, hr:SLAB_R, :],
                        )
                else:
                    xeng.dma_start(
                        out=xbuf[:].rearrange("p a r cc -> p (a r cc)"),
                        in_=xs[:],
                    )
            else:
                # host ships slab A only; build B = A shifted one column
                nc.scalar.dma_start(
                    out=xbuf[:, 0, :, :].rearrange("p r cc -> p (r cc)"),
                    in_=xs[:],
                )
                bdst = xbuf[:, 1, :, 0 : SLAB_C - 1]
                bsrc = xbuf[:, 0, :, 1:SLAB_C]
                if xmode == "sbufb":
                    beng = nc.scalar if bq == "scalar" else nc.sync
                    beng.dma_start(out=bdst, in_=bsrc)
                elif xmode == "gpsb":
                    nc.gpsimd.tensor_copy(bdst, bsrc)
                elif xmode == "dveb":
                    nc.vector.tensor_copy(bdst, bsrc)
                else:
                    raise ValueError(xmode)
            _emit_pass(nc, pool, xbuf, wv, ov, rt=rt, grp=grp, mode=mode,
                       osync=osync, tmpb=tmpb, oq=oq, wsplit=wsplit)


def _xtap(xbuf, r0, rt, t):
    i, j = divmod(t, KW)
    ab, col = (1, 0) if j == 1 else (0, j)
    return xbuf[:, ab, r0 + i : r0 + i + rt, col : col + W]


def _emit_pass(nc, pool, xbuf, wv, ov, rt, grp, mode="full", osync=False,
               tmpb=1, oq=None, wsplit=False):
    for k in range(HALF // rt):
        r0 = k * rt
        if mode == "dma":
            for t in range(KH * KW):
                wt = pool.tile([128, rt, W], _F16, name=f"wg{t % grp}")
                nc.sync.dma_start(out=wt[:], in_=wv[:, :, t, r0 : r0 + rt, :])
            nc.scalar.dma_start(
                out=ov[:, r0 : r0 + rt, :], in_=xbuf[:, 0, r0 : r0 + rt, 0:W]
            )
            continue
        acc = pool.tile([128, rt, W], _F16, name="acc")
        tmp = pool.tile([128, rt, W], _F16, name="tmp", bufs=tmpb)
        first = True
        for g0 in range(0, KH * KW, grp):
            wts = []
            for t in range(g0, min(g0 + grp, KH * KW)):
                wt = pool.tile([128, rt, W], _F16, name=f"wg{t - g0}")
                if mode != "compute":
                    weng = nc.scalar if (wsplit and t % 2 == 1) else nc.sync
                    weng.dma_start(
                        out=wt[:], in_=wv[:, :, t, r0 : r0 + rt, :]
                    )
                wts.append((t, wt))
            for t, wt in wts:
                if first:
                    nc.vector.tensor_tensor(
                        acc[:], _xtap(xbuf, r0, rt, t), wt[:],
                        mybir.AluOpType.mult,
                    )
                    first = False
                else:
                    nc.vector.tensor_tensor(
                        tmp[:], _xtap(xbuf, r0, rt, t), wt[:],
                        mybir.AluOpType.mult,
                    )
                    nc.vector.tensor_tensor(
                        acc[:], acc[:], tmp[:], mybir.AluOpType.add
                    )
        if mode != "compute":
            if oq is None:
                eng = nc.sync if osync else nc.scalar
            else:
                eng = {"sync": nc.sync, "scalar": nc.scalar,
                       "gpsimd": nc.gpsimd}[oq]
            eng.dma_start(out=ov[:, r0 : r0 + rt, :], in_=acc[:])


def build_program(rep=1, rt=None, grp=None, mode="full", xsplit=False,
                  osync=True, tmpb=1, xmode=None, bq="sync", xq="scalar",
                  oq=None, wsplit=False):
    xmode = XMODE if xmode is None else xmode
    nc = bacc.Bacc(
        "TRN2",
        target_bir_lowering=False,
        debug=False,
        enable_asserts=False,
        num_devices=8,
    )
    nslab = 2 if xmode == "dual" else 1
    xs = nc.dram_tensor(
        "xs", [128, nslab * SLAB_R * SLAB_C], _F16, kind="ExternalInput"
    ).ap()
    w = nc.dram_tensor("w", [C, KH, KW, H, W], _F16, kind="ExternalInput").ap()
    o = nc.dram_tensor("o", [C, H, W], _F16, kind="ExternalOutput").ap()
    with TileContext(nc) as tc:
        _emit(nc, tc, xs, w, o, rep=rep, rt=rt, grp=grp, mode=mode,
              xsplit=xsplit, osync=osync, tmpb=tmpb, xmode=xmode, bq=bq,
              xq=xq, oq=oq, wsplit=wsplit)
    nc.compile()
    return nc


def make_slab(x_one, xmode=None):
    """Host-side fp16 slab(s) for one sample: [64,128,128] ->
    [128, {1,2}*66*130].

    Partition p = c*2 + hf holds rows hf*64-1 .. hf*64+64 of channel c
    (zero-padded at the image border) in a 66x130 col-padded layout: slab A
    natural and (xmode=="dual" only) slab B shifted left one column (so the
    j=1 column tap starts 4-byte-aligned and keeps the DVE in 2x_1P packed
    mode; the other xmodes build B on-device).
    """
    xmode = XMODE if xmode is None else xmode
    dual = xmode == "dual"
    nslab = 2 if dual else 1
    xh = np.ascontiguousarray(x_one).astype(np.float16)
    slab = np.zeros((C, 2, nslab, SLAB_R, SLAB_C), dtype=np.float16)
    # half 0: slab rows 1..65 <- x rows 0..64 (row 0 stays zero: top pad)
    slab[:, 0, 0, 1 : HALF + 2, 1 : W + 1] = xh[:, 0 : HALF + 1, :]
    # half 1: slab rows 0..64 <- x rows 63..127 (row 65 stays zero: bottom pad)
    slab[:, 1, 0, 0 : HALF + 1, 1 : W + 1] = xh[:, HALF - 1 : H, :]
    if dual:
        # slab B = slab A shifted left one column
        slab[:, :, 1, :, 0 : SLAB_C - 1] = slab[:, :, 0, :, 1:SLAB_C]
    return slab.reshape(128, nslab * SLAB_R * SLAB_C)


def host_in_full(input, weight, xmode=None):
    """Full-batch host-side input dict keyed by DRAM tensor names, with each
    array laid out as the concat over cores along axis 0 (for shard_map)."""
    return {
        "xs": np.concatenate(
            [make_slab(input[b], xmode) for b in range(B)], axis=0
        ),
        "w": np.ascontiguousarray(weight).astype(np.float16).reshape(
            B * C, KH, KW, H, W
        ),
    }


_CACHE = {}


def _spot_check(out, input, weight, n=512, seed=0):
    """Host-check a random subset of outputs against the fp16-rounded math.

    Catches silent device corruption (cold-NEFF flakes produce garbage far
    outside the ~1e-3 fp16 rounding envelope).
    """
    rng = np.random.default_rng(seed)
    bb = rng.integers(0, B, n)
    cc = rng.integers(0, C, n)
    hh = rng.integers(0, H, n)
    ww = rng.integers(0, W, n)
    xh = input.astype(np.float16)
    whh = weight.astype(np.float16)
    ref = np.zeros(n, np.float64)
    for i in range(KH):
        for j in range(KW):
            h_in = hh + i - 1
            w_in = ww + j - 1
            valid = (h_in >= 0) & (h_in < H) & (w_in >= 0) & (w_in < W)
            xv = np.where(
                valid,
                xh[bb, cc, np.clip(h_in, 0, H - 1), np.clip(w_in, 0, W - 1)],
                np.float16(0),
            ).astype(np.float64)
            ref += xv * whh[bb, cc, i, j, hh, ww].astype(np.float64)
    got = out[bb, cc, hh, ww].astype(np.float64)
    scale = max(np.abs(ref).max(), 1.0)
    return np.abs(got - ref).max() / scale < 5e-3


def kernel(input, weight, _trace=False):
    input = np.asarray(input, dtype=np.float32)
    weight = np.asarray(weight, dtype=np.float32)
    assert input.shape == (B, C, H, W), input.shape
    assert weight.shape == (B, C, KH, KW, H, W), weight.shape

    if "nc" not in _CACHE:
        _CACHE["nc"] = build_program()
    nc = _CACHE["nc"]

    wh = weight.astype(np.float16)
    in_maps = [
        {"xs": make_slab(input[b]), "w": np.ascontiguousarray(wh[b])}
        for b in range(B)
    ]

    def run_once():
        res = run_bass_kernel_spmd(
            nc, in_maps, core_ids=list(range(B)), trace=_trace
        )
        _CACHE["last_result"] = res
        return np.stack([res.results[b]["o"] for b in range(B)], axis=0)

    # Retry both on exceptions (transient NRT/axon hiccups, e.g.
    # NRT_EXEC_UNIT_UNRECOVERABLE on a cold device) and on silent
    # corruption (spot-check failure) -- both observed, both clear on
    # rerun.
    import time as _time

    out = None
    for attempt in range(3):
        try:
            out = run_once()
        except Exception:
            if attempt == 2:
                raise
            _time.sleep(3)
            continue
        if _spot_check(out, input, weight):
            break
        _time.sleep(1)
    return out.astype(np.float32)


# revision 31
# speedup vs baseline: 1.3441x; 1.1834x over previous
"""Dynamic per-pixel depthwise 3x3 conv (DYDConv2d) on 8 Trainium2 cores.

Full-tensor contract:
    input : (8, 64, 128, 128) f32
    weight: (8, 64, 3, 3, 128, 128) f32   -- one 3x3 filter per (b, c, oh, ow)
    out   : (8, 64, 128, 128) f32
    out[b,c,oh,ow] = sum_{i,j} xpad[b,c,oh+i,ow+j] * weight[b,c,i,j,oh,ow]
    (stride 1, pad 1)

Sharding: data-parallel over batch B=8 -> one sample per NeuronCore.

fp16 throughout on device (harness gate is rel_err < 2e-2; measured fp16
error ~9.1e-4): halves the ~38 MB/core weight stream that is the DMA
roofline term AND unlocks the DVE 2x_1P packed perf mode, halving the
vector-engine time of the 17 tensor_tensor ops per output element. Both
roofline terms drop 2x vs the fp32 kernel (measured floors per pass:
DVE ~70 us, HBM DMA ~63-69 us; full pass ~70-74 us vs ~147 us true fp32).

Per-core layout: 128 SBUF partitions = (channel c) x (H-half hf), p =
c*2 + hf. Each partition holds TWO 66x130 fp16 zero-padded slabs of its
half-image: slab A (natural) serving column taps j=0 (byte offset 0) and
j=2 (offset 4), and slab B (= A shifted left one column) serving j=1 at
offset 0. Without B, the j=1 taps start 2 bytes into a 4-byte word and the
DVE silently drops those ops from 2x_1P to 1x packing. The host ships only
slab A; the otherwise-idle Activation engine builds B with a shift-copy
through its own SBUF port (xmode="actb") -- the binding DMA constraint is
total SDMA engine-bytes (~370 GB/s, fabric and HBM alike), so an ACT copy
is the only B build that costs neither SDMA bytes nor DVE cycles. The full
half-image is one rt=64-row chunk: the 9 weight tiles (2 MB DMAs) stream in
groups of 2, double-buffered, while the DVE runs the 17-op mult/add chain
(FD = 8192). Engine split: weights + out-store on the sync HWDGE queue,
x load on the scalar queue, B-copy on ACT. kernel() spot-checks 512 random
outputs on the host and reruns the device pass on mismatch (cold-NEFF
flakes observed).
"""

import numpy as np

import concourse.bacc as bacc
import concourse.mybir as mybir
from concourse.bass_utils import run_bass_kernel_spmd
from concourse.tile import TileContext

B, C, H, W = 8, 64, 128, 128
KH, KW = 3, 3
HALF = H // 2  # rows per half-image (one partition group)
SLAB_R, SLAB_C = HALF + 2, W + 2  # 66 x 130 padded slab per partition

RT = 64   # output rows per chunk (per half); 64 = one chunk per pass
GRP = 2   # weight tiles streamed per group
XMODE = "actb"  # B-slab source. "actb": host ships slab A only; the idle
                # Activation engine builds B (own SBUF port: no SDMA bytes,
                # no DVE cycles) -- beats "dual" (host ships both; +2.2 MB
                # through the ~370 GB/s SDMA engines), "sbufb" (SBUF->SBUF
                # DMA: still SDMA bytes, and one silent-corruption flake),
                # "gpsb" (GPSIMD copy: POOL port contends with DVE), and
                # "dveb" (DVE copy: lengthens the critical DVE chain).

_F16 = mybir.dt.float16


def _emit(nc, tc, xs, w, o, rep=1, rt=None, grp=None, mode="full",
          xsplit=False, osync=False, tmpb=1, xmode=None, bq="sync",
          xq="scalar", oq=None, wsplit=False):
    """Per-core program. xs:[128, 2*66*130] f16, w:[64,3,3,128,128] f16,
    o:[64,128,128] f16.

    rep > 1 repeats the complete pass (x load included) back-to-back in one
    program -- used only for steady-state timing via differencing.
    mode: "full" | "dma" (no DVE ops) | "compute" (no DMAs) -- for
    roofline decomposition.
    """
    rt = RT if rt is None else rt
    grp = GRP if grp is None else grp
    wv = w.rearrange("c kh kw (hf r) ww -> c hf (kh kw) r ww", hf=2)
    ov = o.rearrange("c (hf r) ww -> (c hf) r ww", hf=2)

    with tc.tile_pool(name="work", bufs=2) as pool:
        if mode == "compute":
            # DMAs only in a setup prologue; reps are pure DVE work.
            # Rep-differencing subtracts the prologue.
            xbuf = pool.tile([128, 2, SLAB_R, SLAB_C], _F16, name="xbuf", bufs=1)
            nc.scalar.dma_start(
                out=xbuf[:].rearrange("p a r cc -> p (a r cc)"), in_=xs[:]
            )
            wts = [
                pool.tile([128, rt, W], _F16, name=f"wc{t}", bufs=1)
                for t in range(3)
            ]
            for t in range(3):
                nc.sync.dma_start(out=wts[t][:], in_=wv[:, :, t, 0:rt, :])
            for _r in range(rep):
                for k in range(HALF // rt):
                    r0 = k * rt
                    acc = pool.tile([128, rt, W], _F16, name="acc")
                    tmp = pool.tile([128, rt, W], _F16, name="tmp", bufs=1)
                    first = True
                    for t in range(KH * KW):
                        wt = wts[t % 3]
                        if first:
                            nc.vector.tensor_tensor(
                                acc[:], _xtap(xbuf, r0, rt, t), wt[:],
                                mybir.AluOpType.mult,
                            )
                            first = False
                        else:
                            nc.vector.tensor_tensor(
                                tmp[:], _xtap(xbuf, r0, rt, t), wt[:],
                                mybir.AluOpType.mult,
                            )
                            nc.vector.tensor_tensor(
                                acc[:], acc[:], tmp[:], mybir.AluOpType.add
                            )
                    nc.scalar.dma_start(
                        out=ov[:, r0 : r0 + rt, :], in_=acc[:]
                    )
            return
        xmode = XMODE if xmode is None else xmode
        for _r in range(rep):
            # double-buffered slab: next rep's x load overlaps the previous
            # rep's tail instead of serializing behind it
            xbuf = pool.tile([128, 2, SLAB_R, SLAB_C], _F16, name="xbuf")
            xeng = {"sync": nc.sync, "scalar": nc.scalar,
                    "gpsimd": nc.gpsimd}[xq]
            if xmode == "dual":
                xv = xs.rearrange("p (a r cc) -> p a r cc", a=2, r=SLAB_R)
                if xsplit:
                    # split by half-slab rows so chunk 0 can start once its
                    # region lands (subtile deps), not after the full 4.4MB
                    hr = SLAB_R // 2 + 2
                    for ab in range(2):
                        xeng.dma_start(
                            out=xbuf[:, ab, 0:hr, :], in_=xv[:, ab, 0:hr, :]
                        )
                        xeng.dma_start(
                            out=xbuf[:, ab, hr:SLAB_R, :],
                            in_=xv[:, ab, hr:SLAB_R, :],
                        )
                else:
                    xeng.dma_start(
                        out=xbuf[:].rearrange("p a r cc -> p (a r cc)"),
                        in_=xs[:],
                    )
            else:
                # host ships slab A only; build B = A shifted one column
                nc.scalar.dma_start(
                    out=xbuf[:, 0, :, :].rearrange("p r cc -> p (r cc)"),
                    in_=xs[:],
                )
                bdst = xbuf[:, 1, :, 0 : SLAB_C - 1]
                bsrc = xbuf[:, 0, :, 1:SLAB_C]
                if xmode == "sbufb":
                    beng = nc.scalar if bq == "scalar" else nc.sync
                    beng.dma_start(out=bdst, in_=bsrc)
                elif xmode == "gpsb":
                    nc.gpsimd.tensor_copy(bdst, bsrc)
                elif xmode == "dveb":
                    nc.vector.tensor_copy(bdst, bsrc)
                elif xmode == "actb":
                    # Activation engine: idle, own SBUF port -- the copy
                    # costs no SDMA bytes and no DVE cycles. Taps j=1 read
                    # only B cols 0..W-1, so copy exactly W columns.
                    nc.scalar.copy(
                        xbuf[:, 1, :, 0:W], xbuf[:, 0, :, 1 : W + 1]
                    )
                else:
                    raise ValueError(xmode)
            _emit_pass(nc, pool, xbuf, wv, ov, rt=rt, grp=grp, mode=mode,
                       osync=osync, tmpb=tmpb, oq=oq, wsplit=wsplit)


def _xtap(xbuf, r0, rt, t):
    i, j = divmod(t, KW)
    ab, col = (1, 0) if j == 1 else (0, j)
    return xbuf[:, ab, r0 + i : r0 + i + rt, col : col + W]


def _emit_pass(nc, pool, xbuf, wv, ov, rt, grp, mode="full", osync=False,
               tmpb=1, oq=None, wsplit=False):
    for k in range(HALF // rt):
        r0 = k * rt
        if mode == "dma":
            for t in range(KH * KW):
                wt = pool.tile([128, rt, W], _F16, name=f"wg{t % grp}")
                nc.sync.dma_start(out=wt[:], in_=wv[:, :, t, r0 : r0 + rt, :])
            nc.scalar.dma_start(
                out=ov[:, r0 : r0 + rt, :], in_=xbuf[:, 0, r0 : r0 + rt, 0:W]
            )
            continue
        acc = pool.tile([128, rt, W], _F16, name="acc")
        tmp = pool.tile([128, rt, W], _F16, name="tmp", bufs=tmpb)
        first = True
        for g0 in range(0, KH * KW, grp):
            wts = []
            for t in range(g0, min(g0 + grp, KH * KW)):
                wt = pool.tile([128, rt, W], _F16, name=f"wg{t - g0}")
                if mode != "compute":
                    weng = nc.scalar if (wsplit and t % 2 == 1) else nc.sync
                    weng.dma_start(
                        out=wt[:], in_=wv[:, :, t, r0 : r0 + rt, :]
                    )
                wts.append((t, wt))
            for t, wt in wts:
                if first:
                    nc.vector.tensor_tensor(
                        acc[:], _xtap(xbuf, r0, rt, t), wt[:],
                        mybir.AluOpType.mult,
                    )
                    first = False
                else:
                    nc.vector.tensor_tensor(
                        tmp[:], _xtap(xbuf, r0, rt, t), wt[:],
                        mybir.AluOpType.mult,
                    )
                    nc.vector.tensor_tensor(
                        acc[:], acc[:], tmp[:], mybir.AluOpType.add
                    )
        if mode != "compute":
            if oq is None:
                eng = nc.sync if osync else nc.scalar
            else:
                eng = {"sync": nc.sync, "scalar": nc.scalar,
                       "gpsimd": nc.gpsimd}[oq]
            eng.dma_start(out=ov[:, r0 : r0 + rt, :], in_=acc[:])


def build_program(rep=1, rt=None, grp=None, mode="full", xsplit=False,
                  osync=True, tmpb=1, xmode=None, bq="sync", xq="scalar",
                  oq=None, wsplit=False):
    xmode = XMODE if xmode is None else xmode
    nc = bacc.Bacc(
        "TRN2",
        target_bir_lowering=False,
        debug=False,
        enable_asserts=False,
        num_devices=8,
    )
    nslab = 2 if xmode == "dual" else 1
    xs = nc.dram_tensor(
        "xs", [128, nslab * SLAB_R * SLAB_C], _F16, kind="ExternalInput"
    ).ap()
    w = nc.dram_tensor("w", [C, KH, KW, H, W], _F16, kind="ExternalInput").ap()
    o = nc.dram_tensor("o", [C, H, W], _F16, kind="ExternalOutput").ap()
    with TileContext(nc) as tc:
        _emit(nc, tc, xs, w, o, rep=rep, rt=rt, grp=grp, mode=mode,
              xsplit=xsplit, osync=osync, tmpb=tmpb, xmode=xmode, bq=bq,
              xq=xq, oq=oq, wsplit=wsplit)
    nc.compile()
    return nc


def make_slab(x_one, xmode=None):
    """Host-side fp16 slab(s) for one sample: [64,128,128] ->
    [128, {1,2}*66*130].

    Partition p = c*2 + hf holds rows hf*64-1 .. hf*64+64 of channel c
    (zero-padded at the image border) in a 66x130 col-padded layout: slab A
    natural and (xmode=="dual" only) slab B shifted left one column (so the
    j=1 column tap starts 4-byte-aligned and keeps the DVE in 2x_1P packed
    mode; the other xmodes build B on-device).
    """
    xmode = XMODE if xmode is None else xmode
    dual = xmode == "dual"
    nslab = 2 if dual else 1
    xh = np.ascontiguousarray(x_one).astype(np.float16)
    slab = np.zeros((C, 2, nslab, SLAB_R, SLAB_C), dtype=np.float16)
    # half 0: slab rows 1..65 <- x rows 0..64 (row 0 stays zero: top pad)
    slab[:, 0, 0, 1 : HALF + 2, 1 : W + 1] = xh[:, 0 : HALF + 1, :]
    # half 1: slab rows 0..64 <- x rows 63..127 (row 65 stays zero: bottom pad)
    slab[:, 1, 0, 0 : HALF + 1, 1 : W + 1] = xh[:, HALF - 1 : H, :]
    if dual:
        # slab B = slab A shifted left one column
        slab[:, :, 1, :, 0 : SLAB_C - 1] = slab[:, :, 0, :, 1:SLAB_C]
    return slab.reshape(128, nslab * SLAB_R * SLAB_C)


def host_in_full(input, weight, xmode=None):
    """Full-batch host-side input dict keyed by DRAM tensor names, with each
    array laid out as the concat over cores along axis 0 (for shard_map)."""
    return {
        "xs": np.concatenate(
            [make_slab(input[b], xmode) for b in range(B)], axis=0
        ),
        "w": np.ascontiguousarray(weight).astype(np.float16).reshape(
            B * C, KH, KW, H, W
        ),
    }


_CACHE = {}


def _spot_check(out, input, weight, n=512, seed=0):
    """Host-check a random subset of outputs against the fp16-rounded math.

    Catches silent device corruption (cold-NEFF flakes produce garbage far
    outside the ~1e-3 fp16 rounding envelope).
    """
    rng = np.random.default_rng(seed)
    bb = rng.integers(0, B, n)
    cc = rng.integers(0, C, n)
    hh = rng.integers(0, H, n)
    ww = rng.integers(0, W, n)
    xh = input.astype(np.float16)
    whh = weight.astype(np.float16)
    ref = np.zeros(n, np.float64)
    for i in range(KH):
        for j in range(KW):
            h_in = hh + i - 1
            w_in = ww + j - 1
            valid = (h_in >= 0) & (h_in < H) & (w_in >= 0) & (w_in < W)
            xv = np.where(
                valid,
                xh[bb, cc, np.clip(h_in, 0, H - 1), np.clip(w_in, 0, W - 1)],
                np.float16(0),
            ).astype(np.float64)
            ref += xv * whh[bb, cc, i, j, hh, ww].astype(np.float64)
    got = out[bb, cc, hh, ww].astype(np.float64)
    scale = max(np.abs(ref).max(), 1.0)
    return np.abs(got - ref).max() / scale < 5e-3


def kernel(input, weight, _trace=False):
    input = np.asarray(input, dtype=np.float32)
    weight = np.asarray(weight, dtype=np.float32)
    assert input.shape == (B, C, H, W), input.shape
    assert weight.shape == (B, C, KH, KW, H, W), weight.shape

    if "nc" not in _CACHE:
        _CACHE["nc"] = build_program()
    nc = _CACHE["nc"]

    wh = weight.astype(np.float16)
    in_maps = [
        {"xs": make_slab(input[b]), "w": np.ascontiguousarray(wh[b])}
        for b in range(B)
    ]

    def run_once():
        res = run_bass_kernel_spmd(
            nc, in_maps, core_ids=list(range(B)), trace=_trace
        )
        _CACHE["last_result"] = res
        return np.stack([res.results[b]["o"] for b in range(B)], axis=0)

    # Retry both on exceptions (transient NRT/axon hiccups, e.g.
    # NRT_EXEC_UNIT_UNRECOVERABLE on a cold device) and on silent
    # corruption (spot-check failure) -- both observed, both clear on
    # rerun.
    import time as _time

    out = None
    for attempt in range(3):
        try:
            out = run_once()
        except Exception:
            if attempt == 2:
                raise
            _time.sleep(3)
            continue
        if _spot_check(out, input, weight):
            break
        _time.sleep(1)
    return out.astype(np.float32)


# revision 34
# speedup vs baseline: 1.3937x; 1.0369x over previous
"""Dynamic per-pixel depthwise 3x3 conv (DYDConv2d) on 8 Trainium2 cores.

Full-tensor contract:
    input : (8, 64, 128, 128) f32
    weight: (8, 64, 3, 3, 128, 128) f32   -- one 3x3 filter per (b, c, oh, ow)
    out   : (8, 64, 128, 128) f32
    out[b,c,oh,ow] = sum_{i,j} xpad[b,c,oh+i,ow+j] * weight[b,c,i,j,oh,ow]
    (stride 1, pad 1)

Sharding: data-parallel over batch B=8 -> one sample per NeuronCore.

fp16 throughout on device (harness gate is rel_err < 2e-2; measured fp16
error ~9.1e-4): halves the ~38 MB/core weight stream that is the DMA
roofline term AND unlocks the DVE 2x_1P packed perf mode, halving the
vector-engine time of the 17 tensor_tensor ops per output element. Both
roofline terms drop 2x vs the fp32 kernel (measured floors per pass:
DVE ~70 us, HBM DMA ~63-69 us; full pass ~70-74 us vs ~147 us true fp32).

Per-core layout: 128 SBUF partitions = (channel c) x (H-half hf), p =
c*2 + hf. Each partition holds TWO 66x130 fp16 zero-padded slabs of its
half-image: slab A (natural) serving column taps j=0 (byte offset 0) and
j=2 (offset 4), and slab B (= A shifted left one column) serving j=1 at
offset 0. Without B, the j=1 taps start 2 bytes into a 4-byte word and the
DVE silently drops those ops from 2x_1P to 1x packing. The host ships only
slab A; the otherwise-idle Activation engine builds B with a shift-copy
through its own SBUF port (xmode="actb") -- the binding DMA constraint is
total SDMA engine-bytes (~370 GB/s, fabric and HBM alike), so an ACT copy
is the only B build that costs neither SDMA bytes nor DVE cycles. The full
half-image is one rt=64-row chunk: the 9 weight tiles (2 MB DMAs) stream in
groups of 2, double-buffered, while the DVE runs the 17-op mult/add chain
(FD = 8192). Engine split: weights + out-store on the sync HWDGE queue,
x load on the scalar queue, B-copy on ACT. kernel() spot-checks 512 random
outputs on the host and reruns the device pass on mismatch (cold-NEFF
flakes observed).

Next step if revisited (not landed -- needs hours, found with minutes left):
clear-window full passes hit 55.8 us = 23.2 MB at ~416 GB/s, i.e. DMA-bound
with the in-situ DVE below it (isolated DVE probes measure ~74-76 us but are
bank-conflict artifacts; the full kernel is ground truth). So shipping a
subset of the 9 weight taps as fp8_e4m3 (est. rel err ~7e-3 for 4 taps, vs
the 2e-2 gate) with ACT-side upcast to fp16 would cut the weight stream up
to 2x and could reach ~40-50 us, bounded by the true DVE time.
"""

import numpy as np

import concourse.bacc as bacc
import concourse.mybir as mybir
from concourse.bass_utils import run_bass_kernel_spmd
from concourse.tile import TileContext

B, C, H, W = 8, 64, 128, 128
KH, KW = 3, 3
HALF = H // 2  # rows per half-image (one partition group)
SLAB_R, SLAB_C = HALF + 2, W + 2  # 66 x 130 padded slab per partition

RT = 64   # output rows per chunk (per half); 64 = one chunk per pass
GRP = 2   # weight tiles streamed per group
XMODE = "actb"  # B-slab source. "actb": host ships slab A only; the idle
                # Activation engine builds B (own SBUF port: no SDMA bytes,
                # no DVE cycles) -- beats "dual" (host ships both; +2.2 MB
                # through the ~370 GB/s SDMA engines), "sbufb" (SBUF->SBUF
                # DMA: still SDMA bytes, and one silent-corruption flake),
                # "gpsb" (GPSIMD copy: POOL port contends with DVE), and
                # "dveb" (DVE copy: lengthens the critical DVE chain).

_F16 = mybir.dt.float16


def _emit(nc, tc, xs, w, o, rep=1, rt=None, grp=None, mode="full",
          xsplit=False, osync=False, tmpb=1, xmode=None, bq="sync",
          xq="scalar", oq=None, wsplit=False):
    """Per-core program. xs:[128, 2*66*130] f16, w:[64,3,3,128,128] f16,
    o:[64,128,128] f16.

    rep > 1 repeats the complete pass (x load included) back-to-back in one
    program -- used only for steady-state timing via differencing.
    mode: "full" | "dma" (no DVE ops) | "compute" (no DMAs) -- for
    roofline decomposition.
    """
    rt = RT if rt is None else rt
    grp = GRP if grp is None else grp
    wv = w.rearrange("c kh kw (hf r) ww -> c hf (kh kw) r ww", hf=2)
    ov = o.rearrange("c (hf r) ww -> (c hf) r ww", hf=2)

    with tc.tile_pool(name="work", bufs=2) as pool:
        if mode == "compute9":
            # clean DVE-floor probe: rt=32, NINE distinct weight tiles (no
            # recycled-tile SBUF bank conflicts), DMAs only in the prologue
            rt = 32
            xbuf = pool.tile([128, 2, SLAB_R, SLAB_C], _F16, name="xbuf", bufs=1)
            for ab in range(2):  # timing probe: same data in both slabs
                nc.scalar.dma_start(
                    out=xbuf[:, ab, :, :].rearrange("p r cc -> p (r cc)"),
                    in_=xs[:, 0 : SLAB_R * SLAB_C],
                )
            wts = [
                pool.tile([128, rt, W], _F16, name=f"wn{t}", bufs=1)
                for t in range(KH * KW)
            ]
            for t in range(KH * KW):
                nc.sync.dma_start(out=wts[t][:], in_=wv[:, :, t, 0:rt, :])
            for _r in range(rep):
                for k in range(HALF // rt):
                    r0 = k * rt
                    acc = pool.tile([128, rt, W], _F16, name="acc")
                    tmp = pool.tile([128, rt, W], _F16, name="tmp", bufs=1)
                    first = True
                    for t in range(KH * KW):
                        if first:
                            nc.vector.tensor_tensor(
                                acc[:], _xtap(xbuf, r0, rt, t), wts[t][:],
                                mybir.AluOpType.mult,
                            )
                            first = False
                        else:
                            nc.vector.tensor_tensor(
                                tmp[:], _xtap(xbuf, r0, rt, t), wts[t][:],
                                mybir.AluOpType.mult,
                            )
                            nc.vector.tensor_tensor(
                                acc[:], acc[:], tmp[:], mybir.AluOpType.add
                            )
                    nc.scalar.dma_start(
                        out=ov[:, r0 : r0 + rt, :], in_=acc[:]
                    )
            return
        if mode == "compute":
            # DMAs only in a setup prologue; reps are pure DVE work.
            # Rep-differencing subtracts the prologue.
            xbuf = pool.tile([128, 2, SLAB_R, SLAB_C], _F16, name="xbuf", bufs=1)
            nc.scalar.dma_start(
                out=xbuf[:].rearrange("p a r cc -> p (a r cc)"), in_=xs[:]
            )
            wts = [
                pool.tile([128, rt, W], _F16, name=f"wc{t}", bufs=1)
                for t in range(3)
            ]
            for t in range(3):
                nc.sync.dma_start(out=wts[t][:], in_=wv[:, :, t, 0:rt, :])
            for _r in range(rep):
                for k in range(HALF // rt):
                    r0 = k * rt
                    acc = pool.tile([128, rt, W], _F16, name="acc")
                    tmp = pool.tile([128, rt, W], _F16, name="tmp", bufs=1)
                    first = True
                    for t in range(KH * KW):
                        wt = wts[t % 3]
                        if first:
                            nc.vector.tensor_tensor(
                                acc[:], _xtap(xbuf, r0, rt, t), wt[:],
                                mybir.AluOpType.mult,
                            )
                            first = False
                        else:
                            nc.vector.tensor_tensor(
                                tmp[:], _xtap(xbuf, r0, rt, t), wt[:],
                                mybir.AluOpType.mult,
                            )
                            nc.vector.tensor_tensor(
                                acc[:], acc[:], tmp[:], mybir.AluOpType.add
                            )
                    nc.scalar.dma_start(
                        out=ov[:, r0 : r0 + rt, :], in_=acc[:]
                    )
            return
        xmode = XMODE if xmode is None else xmode
        for _r in range(rep):
            # double-buffered slab: next rep's x load overlaps the previous
            # rep's tail instead of serializing behind it
            xbuf = pool.tile([128, 2, SLAB_R, SLAB_C], _F16, name="xbuf")
            xeng = {"sync": nc.sync, "scalar": nc.scalar,
                    "gpsimd": nc.gpsimd}[xq]
            if xmode == "dual":
                xv = xs.rearrange("p (a r cc) -> p a r cc", a=2, r=SLAB_R)
                if xsplit:
                    # split by half-slab rows so chunk 0 can start once its
                    # region lands (subtile deps), not after the full 4.4MB
                    hr = SLAB_R // 2 + 2
                    for ab in range(2):
                        xeng.dma_start(
                            out=xbuf[:, ab, 0:hr, :], in_=xv[:, ab, 0:hr, :]
                        )
                        xeng.dma_start(
                            out=xbuf[:, ab, hr:SLAB_R, :],
                            in_=xv[:, ab, hr:SLAB_R, :],
                        )
                else:
                    xeng.dma_start(
                        out=xbuf[:].rearrange("p a r cc -> p (a r cc)"),
                        in_=xs[:],
                    )
            else:
                # host ships slab A only; build B = A shifted one column
                nc.scalar.dma_start(
                    out=xbuf[:, 0, :, :].rearrange("p r cc -> p (r cc)"),
                    in_=xs[:],
                )
                bdst = xbuf[:, 1, :, 0 : SLAB_C - 1]
                bsrc = xbuf[:, 0, :, 1:SLAB_C]
                if xmode == "sbufb":
                    beng = nc.scalar if bq == "scalar" else nc.sync
                    beng.dma_start(out=bdst, in_=bsrc)
                elif xmode == "gpsb":
                    nc.gpsimd.tensor_copy(bdst, bsrc)
                elif xmode == "dveb":
                    nc.vector.tensor_copy(bdst, bsrc)
                elif xmode == "actb":
                    # Activation engine: idle, own SBUF port -- the copy
                    # costs no SDMA bytes and no DVE cycles. Taps j=1 read
                    # only B cols 0..W-1, so copy exactly W columns.
                    nc.scalar.copy(
                        xbuf[:, 1, :, 0:W], xbuf[:, 0, :, 1 : W + 1]
                    )
                else:
                    raise ValueError(xmode)
            _emit_pass(nc, pool, xbuf, wv, ov, rt=rt, grp=grp, mode=mode,
                       osync=osync, tmpb=tmpb, oq=oq, wsplit=wsplit)


def _xtap(xbuf, r0, rt, t):
    i, j = divmod(t, KW)
    ab, col = (1, 0) if j == 1 else (0, j)
    return xbuf[:, ab, r0 + i : r0 + i + rt, col : col + W]


def _emit_pass(nc, pool, xbuf, wv, ov, rt, grp, mode="full", osync=False,
               tmpb=1, oq=None, wsplit=False):
    for k in range(HALF // rt):
        r0 = k * rt
        if mode == "dma":
            for t in range(KH * KW):
                wt = pool.tile([128, rt, W], _F16, name=f"wg{t % grp}")
                nc.sync.dma_start(out=wt[:], in_=wv[:, :, t, r0 : r0 + rt, :])
            nc.scalar.dma_start(
                out=ov[:, r0 : r0 + rt, :], in_=xbuf[:, 0, r0 : r0 + rt, 0:W]
            )
            continue
        acc = pool.tile([128, rt, W], _F16, name="acc")
        tmp = pool.tile([128, rt, W], _F16, name="tmp", bufs=tmpb)
        first = True
        for g0 in range(0, KH * KW, grp):
            wts = []
            for t in range(g0, min(g0 + grp, KH * KW)):
                wt = pool.tile([128, rt, W], _F16, name=f"wg{t - g0}")
                if mode != "compute":
                    weng = nc.scalar if (wsplit and t % 2 == 1) else nc.sync
                    weng.dma_start(
                        out=wt[:], in_=wv[:, :, t, r0 : r0 + rt, :]
                    )
                wts.append((t, wt))
            for t, wt in wts:
                if first:
                    nc.vector.tensor_tensor(
                        acc[:], _xtap(xbuf, r0, rt, t), wt[:],
                        mybir.AluOpType.mult,
                    )
                    first = False
                else:
                    nc.vector.tensor_tensor(
                        tmp[:], _xtap(xbuf, r0, rt, t), wt[:],
                        mybir.AluOpType.mult,
                    )
                    nc.vector.tensor_tensor(
                        acc[:], acc[:], tmp[:], mybir.AluOpType.add
                    )
        if mode != "compute":
            if oq is None:
                eng = nc.sync if osync else nc.scalar
            else:
                eng = {"sync": nc.sync, "scalar": nc.scalar,
                       "gpsimd": nc.gpsimd}[oq]
            eng.dma_start(out=ov[:, r0 : r0 + rt, :], in_=acc[:])


def build_program(rep=1, rt=None, grp=None, mode="full", xsplit=False,
                  osync=True, tmpb=1, xmode=None, bq="sync", xq="scalar",
                  oq=None, wsplit=False):
    xmode = XMODE if xmode is None else xmode
    nc = bacc.Bacc(
        "TRN2",
        target_bir_lowering=False,
        debug=False,
        enable_asserts=False,
        num_devices=8,
    )
    nslab = 2 if xmode == "dual" else 1
    xs = nc.dram_tensor(
        "xs", [128, nslab * SLAB_R * SLAB_C], _F16, kind="ExternalInput"
    ).ap()
    w = nc.dram_tensor("w", [C, KH, KW, H, W], _F16, kind="ExternalInput").ap()
    o = nc.dram_tensor("o", [C, H, W], _F16, kind="ExternalOutput").ap()
    with TileContext(nc) as tc:
        _emit(nc, tc, xs, w, o, rep=rep, rt=rt, grp=grp, mode=mode,
              xsplit=xsplit, osync=osync, tmpb=tmpb, xmode=xmode, bq=bq,
              xq=xq, oq=oq, wsplit=wsplit)
    nc.compile()
    return nc


def make_slab(x_one, xmode=None):
    """Host-side fp16 slab(s) for one sample: [64,128,128] ->
    [128, {1,2}*66*130].

    Partition p = c*2 + hf holds rows hf*64-1 .. hf*64+64 of channel c
    (zero-padded at the image border) in a 66x130 col-padded layout: slab A
    natural and (xmode=="dual" only) slab B shifted left one column (so the
    j=1 column tap starts 4-byte-aligned and keeps the DVE in 2x_1P packed
    mode; the other xmodes build B on-device).
    """
    xmode = XMODE if xmode is None else xmode
    dual = xmode == "dual"
    nslab = 2 if dual else 1
    xh = np.ascontiguousarray(x_one).astype(np.float16)
    slab = np.zeros((C, 2, nslab, SLAB_R, SLAB_C), dtype=np.float16)
    # half 0: slab rows 1..65 <- x rows 0..64 (row 0 stays zero: top pad)
    slab[:, 0, 0, 1 : HALF + 2, 1 : W + 1] = xh[:, 0 : HALF + 1, :]
    # half 1: slab rows 0..64 <- x rows 63..127 (row 65 stays zero: bottom pad)
    slab[:, 1, 0, 0 : HALF + 1, 1 : W + 1] = xh[:, HALF - 1 : H, :]
    if dual:
        # slab B = slab A shifted left one column
        slab[:, :, 1, :, 0 : SLAB_C - 1] = slab[:, :, 0, :, 1:SLAB_C]
    return slab.reshape(128, nslab * SLAB_R * SLAB_C)


def host_in_full(input, weight, xmode=None):
    """Full-batch host-side input dict keyed by DRAM tensor names, with each
    array laid out as the concat over cores along axis 0 (for shard_map)."""
    return {
        "xs": np.concatenate(
            [make_slab(input[b], xmode) for b in range(B)], axis=0
        ),
        "w": np.ascontiguousarray(weight).astype(np.float16).reshape(
            B * C, KH, KW, H, W
        ),
    }


_CACHE = {}


def _spot_check(out, input, weight, n=512, seed=0):
    """Host-check a random subset of outputs against the fp16-rounded math.

    Catches silent device corruption (cold-NEFF flakes produce garbage far
    outside the ~1e-3 fp16 rounding envelope).
    """
    rng = np.random.default_rng(seed)
    bb = rng.integers(0, B, n)
    cc = rng.integers(0, C, n)
    hh = rng.integers(0, H, n)
    ww = rng.integers(0, W, n)
    xh = input.astype(np.float16)
    whh = weight.astype(np.float16)
    ref = np.zeros(n, np.float64)
    for i in range(KH):
        for j in range(KW):
            h_in = hh + i - 1
            w_in = ww + j - 1
            valid = (h_in >= 0) & (h_in < H) & (w_in >= 0) & (w_in < W)
            xv = np.where(
                valid,
                xh[bb, cc, np.clip(h_in, 0, H - 1), np.clip(w_in, 0, W - 1)],
                np.float16(0),
            ).astype(np.float64)
            ref += xv * whh[bb, cc, i, j, hh, ww].astype(np.float64)
    got = out[bb, cc, hh, ww].astype(np.float64)
    scale = max(np.abs(ref).max(), 1.0)
    return np.abs(got - ref).max() / scale < 5e-3


def kernel(input, weight, _trace=False):
    input = np.asarray(input, dtype=np.float32)
    weight = np.asarray(weight, dtype=np.float32)
    assert input.shape == (B, C, H, W), input.shape
    assert weight.shape == (B, C, KH, KW, H, W), weight.shape

    if "nc" not in _CACHE:
        _CACHE["nc"] = build_program()
    nc = _CACHE["nc"]

    wh = weight.astype(np.float16)
    in_maps = [
        {"xs": make_slab(input[b]), "w": np.ascontiguousarray(wh[b])}
        for b in range(B)
    ]

    def run_once():
        res = run_bass_kernel_spmd(
            nc, in_maps, core_ids=list(range(B)), trace=_trace
        )
        _CACHE["last_result"] = res
        return np.stack([res.results[b]["o"] for b in range(B)], axis=0)

    # Retry both on exceptions (transient NRT/axon hiccups, e.g.
    # NRT_EXEC_UNIT_UNRECOVERABLE on a cold device) and on silent
    # corruption (spot-check failure) -- both observed, both clear on
    # rerun.
    import time as _time

    out = None
    for attempt in range(3):
        try:
            out = run_once()
        except Exception:
            if attempt == 2:
                raise
            _time.sleep(3)
            continue
        if _spot_check(out, input, weight):
            break
        _time.sleep(1)
    return out.astype(np.float32)
